# revision 47
# baseline (speedup 1.0000x reference)
"""Trainium2 Bass kernel for nn_LocalAttention (Luong local attention, N=64, L=H=1024).

Strategy
--------
Data-parallel over batch: 8 batches per NeuronCore x 8 cores.

Host-side layout prep (no model FLOPs on host):
  * For each batch n, p_t = max(src_len - time_step, -1). The Gaussian
    exp(-(l-p_t)^2/25) underflows to exactly 0.0f for |l-p_t| > 51, so the
    context reduction only needs a 128-wide window around p_t.
  * We ROLL each batch's source axis so that window lands at static slots
    [0, 128). Softmax (max/sum) is permutation-invariant, so scores/softmax
    computed in rolled coordinates are exact. Host passes rolled, transposed
    E^T (h on partitions) for the scores contraction, plus the first 128
    rolled rows as-is (eWin, l on partitions) for the context contraction.
  * W_a is passed as an fp16 hi/lo pair (W ~= hi + 2^-11 lo to ~22 mantissa
    bits): qa errors are amplified by sqrt(H) in the scores dot, so plain
    fp16 W_a would be too coarse. The lo operand of the pairing matmul is
    h/2^11 (host-prescaled) so both partial products accumulate into one
    fp32 PSUM group. W_c is fp16 (tolerance 2e-2).

Device per core (PSUM accumulates fp32 everywhere):
  qa rows = h^T [W_a_hi | W_a_lo]              (PE fp16, streams W_a halves)
  qa^T via 8 tiny PE transposes                (PE fp16)
  per batch b:
    scores = qa_b . E_b^T                      (PE fp16, streams E^T)
    softmax on scores (1,1024) @ partition 0   (DVE max / ACT exp+sum / DVE)
    w = softmax * gauss / Z on window          (DVE fused, then fp16 copy)
    w^T via K=1 matmul with ones               (PE)
    context^T = eWin-chunks^T @ w^T            (PE fp16, 8 tiny matmuls)
  OUT = tanh([context; output] @ W_c^T)        (PE fp16 batched over 8; the
    output@W_c2 half is accumulated mid-stream, context@W_c1 at the tail)
"""

import os
import sys

import numpy as np

for _p in ("/opt/trn_rl_repo", "/root/.axon_site/_ro/trn_rl_repo"):
    if os.path.isdir(_p) and _p not in sys.path:
        sys.path.insert(0, _p)

N, L, H = 64, 1024, 1024
NCORES = 8
NB = N // NCORES  # batches per core
WIN = 128         # static window width after roll
DEV_POW = 25.0
KC = H // 128     # 8 contraction chunks
LO_SCALE = 2.0 ** 11

_PROGRAM = None


def _build_program():
    import concourse.tile as tile
    from concourse import bacc, mybir
    from concourse.bass import MemorySpace, ts
    from concourse.masks import make_identity
    from contextlib import ExitStack

    F32 = mybir.dt.float32
    F16 = mybir.dt.float16
    AF = mybir.ActivationFunctionType
    ALU = mybir.AluOpType

    nc = bacc.Bacc("TRN2", target_bir_lowering=False, debug=False, num_devices=NCORES)
    # eT pre-interleaved on host: [b, p, c*L+l] = E^T[b][128*c+p, l]
    # so every DMA is one contiguous 16KB read per partition.
    eT = nc.dram_tensor("eT", [NB, 128, KC * L], F16, kind="ExternalInput").ap()
    ewin = nc.dram_tensor("ewin", [NB, WIN, H], F16, kind="ExternalInput").ap()
    gauss = nc.dram_tensor("gauss", [1, NB * WIN], F32, kind="ExternalInput").ap()
    # outT16 = fp16(h); outTlo = fp16(2^-11 h) pairs with wa2's lo plane so the
    # lo partial products accumulate into the same PSUM group as the hi ones.
    outT16 = nc.dram_tensor("outT16", [H, NB], F16, kind="ExternalInput").ap()
    outTlo = nc.dram_tensor("outTlo", [H, NB], F16, kind="ExternalInput").ap()
    # wa2[hh, p, c, {hi,lo}, u] = W_pair[128c + p, 512hh + u]
    wa2 = nc.dram_tensor("wa2", [2, 128, KC, 2, 512], F16, kind="ExternalInput").ap()
    wcT = nc.dram_tensor("wcT", [128, 2 * KC, H], F16, kind="ExternalInput").ap()
    res = nc.dram_tensor("res", [NB, H], F32, kind="ExternalOutput").ap()

    with tile.TileContext(nc) as tc, ExitStack() as ctx:
        consts = ctx.enter_context(tc.tile_pool(name="consts", bufs=1))
        # wa and et tiles share one pool (same 16KB/partition footprint): the
        # two wa slots free up after qa and recycle as deep et prefetch slots
        etp = ctx.enter_context(tc.tile_pool(name="etp", bufs=5))
        ewp = ctx.enter_context(tc.tile_pool(name="ewp", bufs=6))
        work = ctx.enter_context(tc.tile_pool(name="work", bufs=2))
        # 3 scores buffers: with only 2, scores(k+2) is gated by exp(k) and any
        # transient chain slip compounds into a fully serial steady state
        ps_s = ctx.enter_context(
            tc.tile_pool(name="ps_s", bufs=3, space=MemorySpace.PSUM)
        )
        ps_m = ctx.enter_context(
            tc.tile_pool(name="ps_m", bufs=2, space=MemorySpace.PSUM)
        )

        # ---- head DMAs: qa inputs first (critical path), then batch 0/1 ----
        outTr_sb = consts.tile([128, KC, NB], F16)
        nc.sync.dma_start(outTr_sb[:], outT16.rearrange("(c p) b -> p c b", p=128))
        outTlo_sb = consts.tile([128, KC, NB], F16)
        nc.sync.dma_start(outTlo_sb[:], outTlo.rearrange("(c p) b -> p c b", p=128))
        wa_tiles = []
        for hh in range(2):
            t = etp.tile([128, KC, 2, 512], F16, tag="et")
            nc.sync.dma_start(t[:], wa2[hh])
            wa_tiles.append(t)
        gauss_sb = consts.tile([1, NB * WIN], F32)
        nc.sync.dma_start(gauss_sb[:], gauss[:])
        et_tiles = {}
        ewin_tiles = {}
        for b in range(3):
            t = etp.tile([128, KC, L], F16, tag="et")
            nc.sync.dma_start(t[:], eT[b].rearrange("p (c l) -> p c l", l=L))
            et_tiles[b] = t
            t = ewp.tile([WIN, H], F16, tag="ewin")
            nc.sync.dma_start(t[:], ewin[b])
            ewin_tiles[b] = t

        ident = consts.tile([128, 128], F16)
        make_identity(nc, ident[:])
        ones1 = consts.tile([1, 1], F16)
        nc.gpsimd.memset(ones1[:], 1.0)
        qaT_sb = consts.tile([128, KC, NB], F16)
        ctxAll = consts.tile([128, KC, NB], F16)

        # ---- qa rows = h^T W_a: hi and (pre-scaled) lo partial products all
        # accumulate into one fp32 PSUM group; one wa half-tile per hh ----
        qrow16 = consts.tile([NB, H], F16)
        for hh in range(2):
            ps_qa = ps_m.tile([NB, 512], F32, tag="misc")
            for t in range(2):
                for c in range(KC):
                    nc.tensor.matmul(
                        ps_qa[:],
                        (outTr_sb if t == 0 else outTlo_sb)[:, c, :],
                        wa_tiles[hh][:, c, t, :],
                        start=(t == 0 and c == 0),
                        stop=(t == 1 and c == KC - 1),
                    )
            nc.vector.tensor_copy(qrow16[:, ts(hh, 512)], ps_qa[:])
            for cc in range(KC // 2):
                c = hh * (KC // 2) + cc
                ps_t = ps_m.tile([128, NB], F16, tag="misc")
                nc.tensor.transpose(ps_t[:], qrow16[:, ts(c, 128)], ident[0:NB, 0:NB])
                nc.vector.tensor_copy(qaT_sb[:, c, :], ps_t[:])

        wcT_sb = consts.tile([128, 2 * KC, H], F16)

        # two-stage pipelined ctx: wT(b-1) issues behind scores(b) (2 waiting
        # instrs fit the 4-deep engine wait queue), the 9-instr ctx(b-2) only
        # issues once its inputs are long ready — otherwise it overflows the
        # wait queue and blocks dispatch of ready scores matmuls behind it.
        wv_state = {}  # b -> wv16 tile
        ew_state = {}  # b -> ewin tile
        wt_state = {}  # b -> wT_sb tile

        wt_ps = {}   # k -> ps_wT awaiting DVE copy
        ctx_ps = {}  # k -> ps_ctx awaiting DVE copy

        def emit_wt_mm(k):
            """w^T = transpose(wv16) via K=1 matmul (PE part)."""
            ps_wT = ps_m.tile([128, 1], F32, tag="misc")
            nc.tensor.matmul(
                ps_wT[:], wv_state.pop(k)[:], ones1[:], start=True, stop=True
            )
            wt_ps[k] = ps_wT

        def emit_wt_copy(k):
            wT_sb = work.tile([128, 1], F16, tag="wT")
            nc.vector.tensor_copy(wT_sb[:], wt_ps.pop(k)[:])
            wt_state[k] = wT_sb

        def emit_ctx_mm(k):
            """context^T chunks: eWin[:, ts(c,128)]^T @ w^T -> (128 h, 1)."""
            ew = ew_state.pop(k)
            wT_sb = wt_state.pop(k)
            ps_ctx = ps_m.tile([128, NB], F32, tag="misc")
            for c in range(KC):
                nc.tensor.matmul(
                    ps_ctx[:, c : c + 1],
                    ew[:, ts(c, 128)],
                    wT_sb[:],
                    start=True,
                    stop=True,
                )
            ctx_ps[k] = ps_ctx

        def emit_ctx_copy(k):
            nc.vector.tensor_copy(ctxAll[:, :, k], ctx_ps.pop(k)[:])

        for b in range(NB):
            # prefetch batch b+3 (b0-b2 were issued before qa)
            pb = b + 3
            if pb < NB:
                t = ewp.tile([WIN, H], F16, tag="ewin")
                nc.sync.dma_start(t[:], ewin[pb])
                ewin_tiles[pb] = t
                t = etp.tile([128, KC, L], F16, tag="et")
                nc.sync.dma_start(t[:], eT[pb].rearrange("p (c l) -> p c l", l=L))
                et_tiles[pb] = t
            if 2 <= b <= 5:
                # wcT quarters trickled behind the eT stream: h-half (d=8..15)
                # first for the pre-tail projection, ctx-half before the tail.
                q = (2, 3, 0, 1)[b - 2]
                nc.sync.dma_start(
                    wcT_sb[:, ts(q, 2 * KC // 4), :], wcT[:, ts(q, 2 * KC // 4), :]
                )

            ps_scores = ps_s.tile([1, L], F32, tag="scores")
            et = et_tiles.pop(b)
            for c in range(KC):
                for hh in range(2):
                    nc.tensor.matmul(
                        ps_scores[:, ts(hh, 512)],
                        qaT_sb[:, c, b : b + 1],
                        et[:, c, ts(hh, 512)],
                        start=(c == 0),
                        stop=(c == KC - 1),
                    )

            # lag-3 ctx / lag-2 wT: deps were satisfied a full batch ago, so
            # the in-order PE never stalls here waiting on a softmax chain
            if b >= 3:
                emit_ctx_mm(b - 3)
            if b >= 2:
                emit_wt_mm(b - 2)

            negmax = work.tile([1, 1], F32, tag="negmax")
            nc.vector.reduce_max(
                negmax[:], ps_scores[:], axis=mybir.AxisListType.X, negate=True
            )
            expv = work.tile([1, L], F32, tag="expv")
            zsum = work.tile([1, 1], F32, tag="zsum")
            nc.scalar.activation(
                expv[:], ps_scores[:], AF.Exp, bias=negmax[:], accum_out=zsum[:]
            )
            rz = work.tile([1, 1], F32, tag="rz")
            nc.vector.reciprocal(rz[:], zsum[:])
            # w (window only) = exp * (1/Z) * gauss
            wv32 = work.tile([1, WIN], F32, tag="wv32")
            nc.vector.scalar_tensor_tensor(
                wv32[:],
                expv[:, 0:WIN],
                rz[:],
                gauss_sb[:, ts(b, WIN)],
                op0=ALU.mult,
                op1=ALU.mult,
            )
            wv16 = work.tile([1, WIN], F16, tag="wv16")
            nc.vector.tensor_copy(wv16[:], wv32[:])
            wv_state[b] = wv16
            ew_state[b] = ewin_tiles.pop(b)

            # DVE copies for the lagged stages go behind this batch's softmax
            # ops: their PE producers finish during/just after scores(b)
            if b >= 3:
                emit_ctx_copy(b - 3)
            if b >= 2:
                emit_wt_copy(b - 2)

        # drain the pipeline
        emit_ctx_mm(NB - 3)
        emit_ctx_copy(NB - 3)
        emit_wt_mm(NB - 2)
        emit_wt_copy(NB - 2)
        emit_ctx_mm(NB - 2)
        emit_ctx_copy(NB - 2)
        emit_wt_mm(NB - 1)
        emit_wt_copy(NB - 1)
        emit_ctx_mm(NB - 1)
        emit_ctx_copy(NB - 1)

        # ---- tail: OUT = tanh(cat @ W_c^T), res in two halves ----
        res_sb = work.tile([NB, H], F32, tag="res")
        for hh in range(2):
            ps_out = ps_m.tile([NB, 512], F32, tag="misc")
            for d in range(2 * KC):
                lhsT = ctxAll[:, d, :] if d < KC else outTr_sb[:, d - KC, :]
                nc.tensor.matmul(
                    ps_out[:],
                    lhsT,
                    wcT_sb[:, d, ts(hh, 512)],
                    start=(d == 0),
                    stop=(d == 2 * KC - 1),
                )
            nc.scalar.activation(res_sb[:, ts(hh, 512)], ps_out[:], AF.Tanh)
            nc.sync.dma_start(res[:, ts(hh, 512)], res_sb[:, ts(hh, 512)])

    nc.compile()
    return nc


def _get_program():
    global _PROGRAM
    if _PROGRAM is None:
        _PROGRAM = _build_program()
    return _PROGRAM


def _prepare(inputs):
    E = np.asarray(inputs["encoder_outputs"], dtype=np.float32)
    out = np.asarray(inputs["output"], dtype=np.float32).reshape(N, H)
    W_a = np.ascontiguousarray(np.asarray(inputs["W_a"], dtype=np.float32))
    W_c = np.asarray(inputs["W_c"], dtype=np.float32)
    src_len = np.asarray(inputs["src_len"]).reshape(N).astype(np.int64)
    t = int(np.asarray(inputs["time_step"]))

    p_t = np.maximum(src_len - t, -1)
    roll = p_t - (WIN // 2 - 1)  # window slot j <-> original l = (j + roll) % L
    j = np.arange(L, dtype=np.int64)
    idx = (j[None, :] + roll[:, None]) % L  # (N, L)
    ptf = p_t.astype(np.float32)[:, None]
    gauss = np.exp(
        -((idx[:, :WIN].astype(np.float32) - ptf) ** 2) / np.float32(DEV_POW)
    ).astype(np.float32)  # (N, WIN)

    Er = E[np.arange(N)[:, None], idx, :]  # (N, L, H) rolled
    ewin_dev = np.ascontiguousarray(Er[:, :WIN, :]).astype(np.float16)  # (N, WIN, H)
    eT = np.ascontiguousarray(Er.transpose(0, 2, 1)).astype(np.float16)  # (N, H, L)
    # interleave for linear per-partition DMA: [n, p, c, l] = eT[n, 128c+p, l]
    eT_dev = np.ascontiguousarray(
        eT.reshape(N, KC, 128, L).transpose(0, 2, 1, 3)
    ).reshape(N, 128, KC * L)
    # W_a fp16 hi/lo pair: W ~= hi + 2^-11 * lo (lo scaled into fp16 range)
    wa_hi = W_a.astype(np.float16)
    wa_lo = ((W_a - wa_hi.astype(np.float32)) * np.float32(LO_SCALE)).astype(np.float16)
    # wa2[hh, p, c, t, u] = pair_t[128c + p, 512hh + u]
    wa2_dev = np.ascontiguousarray(
        np.stack([wa_hi, wa_lo], axis=1)  # (H, 2, H)
        .reshape(KC, 128, 2, 2, 512)
        .transpose(3, 1, 0, 2, 4)
    )
    wcT = np.ascontiguousarray(W_c.T)  # (2H, H)
    wcT_dev = np.ascontiguousarray(
        wcT.reshape(2 * KC, 128, H).transpose(1, 0, 2)
    ).astype(np.float16)  # (128, 2KC, H)

    in_maps = []
    for c in range(NCORES):
        sl = slice(c * NB, (c + 1) * NB)
        outT = np.ascontiguousarray(out[sl].T)
        in_maps.append(
            {
                "eT": eT_dev[sl],
                "ewin": ewin_dev[sl],
                "gauss": np.ascontiguousarray(gauss[sl].reshape(1, NB * WIN)),
                "outT16": outT.astype(np.float16),
                "outTlo": (outT / np.float32(LO_SCALE)).astype(np.float16),
                "wa2": wa2_dev,
                "wcT": wcT_dev,
            }
        )
    return in_maps


def _run(inputs, trace=False, tmpdir=None):
    from concourse.bass_utils import run_bass_kernel_spmd

    nc = _get_program()
    in_maps = _prepare(inputs)
    r = run_bass_kernel_spmd(
        nc, in_maps, core_ids=list(range(NCORES)), trace=trace, tmpdir=tmpdir
    )
    outp = np.concatenate([r.results[c]["res"] for c in range(NCORES)], axis=0)
    return np.ascontiguousarray(outp.reshape(N, 1, H).astype(np.float32)), r


def kernel(**inputs):
    return _run(inputs, trace=False)[0]
